# revision 41
# baseline (speedup 1.0000x reference)
"""Trainium2 Bass kernel for nn_Action_Decoder (GAT-based action decoder).

v2 strategy (8 NeuronCores, pure data-parallel over batch):
  - B=4096 sharded 8 x 512 samples/core; weights replicated; 4 tiles of
    128 samples on the partition dim.
  - Host folds W_proj@W1_obs into one [512,128] block (weight folding
    only), so the obs projection feeds GAT layer 1 directly.
  - Gathers: one combined [node|sub] bf16 table per sample in DRAM; two
    dma_gather(transpose=True) chunks per tile on 8 SWDGE queues, issued
    up-front.
  - Layer-1: h^T = W^T x^T via PE with stationary reuse; the obs+sub
    "shared" part is computed once per tile ([128,128]) and broadcast
    over the 6 nodes with an identity-stationary matmul (0-stride
    moving operand).
  - Fused transpose+e: per node k, ONE matmul with stationary hT_k and
    moving [ident(128) | a_src/a_dst(8)] yields both the batch-layout
    h block and the e_src/e_dst values.
  - All per-sample phases (softmax, apply, elu, layer 2) run batched
    across the 4 tiles to amortize per-instruction overhead; alpha is
    stored (t,i,j,h) so the apply multiplies use a 2D access pattern.
  - elu (exact: exp(min(x,0)) + relu(x) - 1, -1 folded into sum(W2)) +
    layer-2 GAT via affine_mul_reduce + a batched 6x6 attention.
"""

import os
import sys

import numpy as np

for _p in ("/root/.axon_site", "/root/.axon_site/_ro/trn_rl_repo",
           "/root/.axon_site/_ro/pypackages", "/opt/trn_rl_repo", "/opt/pypackages"):
    if os.path.isdir(_p) and _p not in sys.path:
        sys.path.append(_p)

import ml_dtypes

import concourse.bass as bass
import concourse.tile as tile
from concourse import bacc
from concourse import mybir
from concourse.bass_utils import run_bass_kernel_spmd

# Problem dims
B, N, S, K, H, OBS = 4096, 177, 36, 6, 128, 500
HEADS, FH = 4, 32
NCORES = 8
BS = B // NCORES          # 512 samples per core
NT = BS // 128            # 4 tiles of 128 samples
OBS_PAD = 512             # pad 500 -> 512
R = N + S                 # combined table rows per sample (213)

F32 = mybir.dt.float32
BF16 = mybir.dt.bfloat16
I16 = mybir.dt.int16
AX = mybir.AxisListType
OP = mybir.AluOpType
ACT = mybir.ActivationFunctionType

LRELU_SLOPE = 0.2
FOLDS_ON_GPSIMD = False
GPSIMD_MULS = True
DEBUG_DUMP = False


def build_graph(scalars):
    as2 = float(scalars["a_src2"])
    ad2 = float(scalars["a_dst2"])
    b2 = float(scalars["b2"])
    c2 = float(scalars["c2"])

    nc = bacc.Bacc(num_swdge_queues=4)

    obs_T = nc.declare_dram_parameter("obs_T", [128, 4, BS], BF16, isOutput=False)
    comb_emb = nc.declare_dram_parameter("comb_emb", [BS * R, H], BF16, isOutput=False)
    idx16 = nc.declare_dram_parameter("idx16", [16, 256], I16, isOutput=False)
    wfold = nc.declare_dram_parameter("wfold", [128, 4, H], BF16, isOutput=False)
    w1bc = nc.declare_dram_parameter("w1bc", [H, 2, H], BF16, isOutput=False)
    trident = nc.declare_dram_parameter("trident", [H, 136], BF16, isOutput=False)
    identd = nc.declare_dram_parameter("identd", [H, H], BF16, isOutput=False)
    biash = nc.declare_dram_parameter("biash", [H, 1], F32, isOutput=False)
    ce = nc.declare_dram_parameter("ce", [1, 48], F32, isOutput=False)
    w2r = nc.declare_dram_parameter("w2r", [1, H], BF16, isOutput=False)
    out_ext = nc.declare_dram_parameter("out", [BS, K], F32, isOutput=True)
    if DEBUG_DUMP:
        dbg1 = nc.declare_dram_parameter("dbg1", [128, 960], F32, isOutput=True)
        dbg2 = nc.declare_dram_parameter("dbg2", [128, 3072], F32, isOutput=True)

    with tile.TileContext(nc) as tc:
        with (
            tc.tile_pool(name="consts", bufs=1) as consts,
            tc.tile_pool(name="gat", bufs=4) as gat,
            tc.tile_pool(name="ht", bufs=2) as htp,
            tc.tile_pool(name="work", bufs=1) as work,
            tc.tile_pool(name="small", bufs=3) as small,
            tc.tile_pool(name="psH", bufs=2, space="PSUM") as psH,
            tc.tile_pool(name="psS", bufs=1, space="PSUM") as psS,
            tc.tile_pool(name="psT", bufs=1, space="PSUM") as psT,
        ):
            # ---- idx bootstrap: the static DMA rings take ~13us to fill
            # a [128,*] tile (round-robin with the big const DMAs), so load
            # the idx table via a SWDGE gather instead. The channel-index
            # tile (p%16) is built with 16 partition-strided memsets. ----
            chp = consts.tile([128, 8], I16)
            nc.gpsimd.iota(chp[:, :], pattern=[[0, 8]], base=0,
                           channel_multiplier=1)
            ch16 = consts.tile([128, 8], I16)
            nc.vector.tensor_scalar(ch16[:, :], chp[:, :], 15, None,
                                    OP.bitwise_and)
            idx_sb = consts.tile([128, 256], I16)
            nc.gpsimd.dma_gather(
                out_ap=idx_sb[:, :].unsqueeze(1),
                in_ap=idx16[:, :],
                idxs_ap=ch16[:, :],
                num_idxs=128, num_idxs_reg=128, elem_size=256,
                transpose=False, queue_num=0,
            )

            # ---- constants on other queues ----
            wfold_sb = consts.tile([128, 4, 128], BF16)
            nc.sync.dma_start(out=wfold_sb[:, :, :], in_=wfold[:, :, :])
            w1bc_sb = consts.tile([128, 2, 128], BF16)
            nc.scalar.dma_start(out=w1bc_sb[:, :, :], in_=w1bc[:, :, :])
            tri_sb = consts.tile([128, 136], BF16)
            nc.scalar.dma_start(out=tri_sb, in_=trident[:, :])
            ident_sb = consts.tile([128, 128], BF16)
            nc.scalar.dma_start(out=ident_sb, in_=identd[:, :])
            biash_sb = consts.tile([128, 1], F32)
            nc.scalar.dma_start(out=biash_sb, in_=biash[:, :])
            ce_sb = consts.tile([128, 48], F32)
            nc.scalar.dma_start(
                out=ce_sb,
                in_=bass.AP(tensor=ce, offset=0, ap=[[0, 128], [1, 48]]),
            )
            w2_sb = consts.tile([128, 128], BF16)
            nc.scalar.dma_start(
                out=w2_sb,
                in_=bass.AP(tensor=w2r, offset=0, ap=[[0, 128], [1, 128]]),
            )
            obs_sb = consts.tile([128, 4, BS], BF16)
            nc.sync.dma_start(out=obs_sb[:, :, :], in_=obs_T[:, :, :])

            # ---- gathers: 2 chunks per tile, 8 queues, issued up-front ----
            gth = []
            for t in range(NT):
                g = gat.tile([128, 896], BF16, tag=f"g{t}")
                nc.gpsimd.dma_gather(
                    out_ap=g[:, 0:640].rearrange("p (x n) -> p x n", x=1),
                    in_ap=comb_emb[t * 128 * R:(t + 1) * 128 * R, :],
                    idxs_ap=idx_sb[:, t * 56:t * 56 + 40],
                    num_idxs=640, num_idxs_reg=640, elem_size=H,
                    transpose=True, queue_num=(t % 2) * 2 + (t // 2),
                )
                nc.gpsimd.dma_gather(
                    out_ap=g[:, 640:896].rearrange("p (x n) -> p x n", x=1),
                    in_ap=comb_emb[t * 128 * R:(t + 1) * 128 * R, :],
                    idxs_ap=idx_sb[:, t * 56 + 40:t * 56 + 56],
                    num_idxs=256, num_idxs_reg=256, elem_size=H,
                    transpose=True, queue_num=(t % 2) * 2 + 1 - (t // 2),
                )
                gth.append(g)

            # ---- per-tile: shared (obs+sub), node matmuls, broadcast-add,
            # transposes. NOTE: each sh region's 5 accumulating matmuls must
            # be consecutive -- a start=True on the shared PSUM bank clears
            # has_written bits for the WHOLE bank. ----
            hbL = work.tile([128, NT, 768], BF16, tag="hbL")
            e_sd = small.tile([128, NT, 48], F32, tag="esd")
            sh_sb = consts.tile([128, 4, 128], BF16)
            sh_ps = psS.tile([128, 4, 128], F32)
            for t in range(NT):
                g = gth[t]
                # shared = Wfold^T obs^T + W1b^T sub^T (consecutive group)
                for c in range(4):
                    nc.tensor.matmul(
                        sh_ps[:, t, :], wfold_sb[:, c, :],
                        obs_sb[:, c, t * 128:(t + 1) * 128],
                        start=(c == 0), stop=False,
                    )
                nc.tensor.matmul(sh_ps[:, t, :], w1bc_sb[:, 0, :],
                                 g[:, 512:640], start=False, stop=True)
                nc.scalar.activation(sh_sb[:, t, :], sh_ps[:, t, :], ACT.Copy)
                h_ps = psH.tile([128, 768], F32, tag="hps")
                # node part (W1c stationary)
                nc.tensor.matmul(h_ps[:, 0:512], w1bc_sb[:, 1, :], g[:, 0:512],
                                 start=True, stop=False)
                nc.tensor.matmul(h_ps[:, 512:768], w1bc_sb[:, 1, :],
                                 g[:, 640:896], start=True, stop=False)
                # broadcast shared over the 6 nodes (ident stationary,
                # 0-stride moving)
                for hs, nk in ((slice(0, 512), 4), (slice(512, 768), 2)):
                    mov = (sh_sb[:, t, :].unsqueeze(1)
                           .broadcast_to([128, nk, 128]))
                    nc.tensor.matmul(h_ps[:, hs], ident_sb[:, :], mov,
                                     start=False, stop=True)
                # h^T (+bias) -> SBUF bf16
                hT = htp.tile([128, 768], BF16, tag="hT")
                nc.scalar.activation(hT[:, :], h_ps[:, :], ACT.Identity,
                                     bias=biash_sb[:, :], scale=1.0)
                # fused transpose + e: out[:,k,0:128]=hbL block,
                # out[:,k,128:136]=e_src/e_dst. Split 3+3 so no matmul
                # output straddles a PSUM bank (136 f32 x 3 fits in 2KB).
                ps2a = psT.tile([128, 3, 136], F32, tag="ps2a")
                ps2b = psT.tile([128, 3, 136], F32, tag="ps2b")
                for k in range(K):
                    ps2k = ps2a[:, k, :] if k < 3 else ps2b[:, k - 3, :]
                    nc.tensor.matmul(
                        ps2k, hT[:, k * 128:(k + 1) * 128],
                        tri_sb[:, :], start=True, stop=True,
                    )
                # trident's perm already emits (f,h) order: plain evicts
                nc.scalar.activation(
                    hbL[:, t, 0:384].rearrange("p (k f) -> p k f", k=3),
                    ps2a[:, :, 0:128], ACT.Copy)
                nc.scalar.activation(
                    hbL[:, t, 384:768].rearrange("p (k f) -> p k f", k=3),
                    ps2b[:, :, 0:128], ACT.Copy)
                nc.vector.tensor_sub(
                    e_sd[:, t, 0:24].rearrange("p (k s) -> p k s", k=3),
                    ps2a[:, :, 128:136], ce_sb[:, 0:24].rearrange(
                        "p (k s) -> p k s", k=3))
                nc.vector.tensor_sub(
                    e_sd[:, t, 24:48].rearrange("p (k s) -> p k s", k=3),
                    ps2b[:, :, 128:136], ce_sb[:, 24:48].rearrange(
                        "p (k s) -> p k s", k=3))

            # ---- alpha + apply, pipelined over tile PAIRS so the vector
            # phase starts as soon as tiles 0/1 land. Layout (t,i,j,h);
            # apply muls in (j,f,h) (h-innermost step-1 -> DVE 2x);
            # three batched fold adds per half. ----
            e4 = e_sd[:, :, :].rearrange("p t (k s h) -> p t k s h", s=2, h=4)
            e_raw = small.tile([128, NT, 144], BF16, tag="eraw")
            e_lrf = small.tile([128, NT * 144], BF16, tag="elr")
            E1 = small.tile([128, NT * 144], F32, tag="E1")
            Z1 = small.tile([128, 96], F32, tag="Z1")
            rZ1 = small.tile([128, 96], F32, tag="rZ1")
            al = small.tile([128, NT, 144], BF16, tag="al")
            prod = work.tile([128, 24, 768], BF16, tag="prod")
            f1 = work.tile([128, 24, 384], BF16, tag="f1")
            f2 = work.tile([128, 24, 128], BF16, tag="f2")
            attn = work.tile([128, 24, 128], BF16, tag="attn")
            e_rawf = e_raw[:, :, :].rearrange("p t x -> p (t x)")
            for half in range(2):
                th0 = 2 * half
                for t in (th0, th0 + 1):
                    e_dst_ap = (e4[:, t, :, 1, :].unsqueeze(2)
                                .broadcast_to([128, 6, 6, 4]))
                    e_src_ap = (e4[:, t, :, 0, :].unsqueeze(1)
                                .broadcast_to([128, 6, 6, 4]))
                    nc.vector.tensor_tensor(
                        e_raw[:, t, :].rearrange("p (i j h) -> p i j h",
                                                 j=6, h=4),
                        e_dst_ap, e_src_ap, OP.add,
                    )
                sl = slice(th0 * 144, (th0 + 2) * 144)
                zs = slice(half * 48, (half + 1) * 48)
                nc.vector.scalar_tensor_tensor(
                    e_lrf[:, sl], e_rawf[:, sl], LRELU_SLOPE, e_rawf[:, sl],
                    OP.mult, OP.max)
                nc.scalar.activation(E1[:, sl], e_lrf[:, sl], ACT.Exp)
                nc.vector.tensor_reduce(
                    Z1[:, zs],
                    E1[:, sl].rearrange("p (ti j h) -> p ti j h", j=6, h=4)
                    .transpose([0, 1, 3, 2]),
                    axis=AX.X, op=OP.add)
                nc.vector.reciprocal_approx_fast(rZ1[:, zs], Z1[:, zs])
                nc.vector.tensor_tensor(
                    al[:, th0:th0 + 2, :].rearrange(
                        "p t (i j h) -> p (t i) j h", j=6, h=4),
                    E1[:, sl].rearrange("p (ti j h) -> p ti j h", j=6, h=4),
                    rZ1[:, zs].rearrange("p (ti h) -> p ti h", h=4)
                    .unsqueeze(2).broadcast_to([128, 12, 6, 4]),
                    OP.mult,
                )
                for i in range(K):
                    eng = nc.gpsimd if (GPSIMD_MULS and i >= 4) else nc.vector
                    for t in (th0, th0 + 1):
                        eng.tensor_tensor(
                            prod[:, t * 6 + i, :].rearrange(
                                "p (j f h) -> p j f h", f=32, h=4),
                            al[:, t, i * 24:(i + 1) * 24]
                            .rearrange("p (j h) -> p j h", h=4)
                            .unsqueeze(2).broadcast_to([128, 6, 32, 4]),
                            hbL[:, t, :].rearrange("p (j f h) -> p j f h",
                                                   f=32, h=4),
                            OP.mult,
                        )
                cs = slice(th0 * 6, (th0 + 2) * 6)
                nc.vector.tensor_add(f1[:, cs, :], prod[:, cs, 0:384],
                                     prod[:, cs, 384:768])
                nc.vector.tensor_add(f2[:, cs, :], f1[:, cs, 0:128],
                                     f1[:, cs, 128:256])
                nc.vector.tensor_add(attn[:, cs, :], f2[:, cs, :],
                                     f1[:, cs, 256:384])

            # ---- elu(x) = exp(min(x,0)) + relu(x) - 1 (-1 folded into c2)
            attn_f = attn[:, :, :].rearrange("p c f -> p (c f)")
            min_x = work.tile([128, NT * 768], BF16, tag="minx")
            exp_m = work.tile([128, NT * 768], BF16, tag="expm")
            relu_x = work.tile([128, NT * 768], BF16, tag="relux")
            v1 = work.tile([128, NT * 768], BF16, tag="v1")
            HB = NT * 384
            for hh in range(2):
                es = slice(hh * HB, (hh + 1) * HB)
                nc.vector.tensor_scalar_min(min_x[:, es], attn_f[:, es], 0.0)
                nc.scalar.activation(exp_m[:, es], min_x[:, es], ACT.Exp)
                nc.vector.tensor_scalar_max(relu_x[:, es], attn_f[:, es], 0.0)
                nc.vector.tensor_add(v1[:, es], exp_m[:, es], relu_x[:, es])

            if DEBUG_DUMP:
                # dbg1: e_sd (192) | Z1 (96) | rZ1 (96) | al (576, as f32)
                dbg1_sb = work.tile([128, 960], F32, tag="dbg1sb")
                nc.vector.tensor_copy(dbg1_sb[:, 0:192],
                                      e_sd[:, :, :].rearrange("p t x -> p (t x)"))
                nc.vector.tensor_copy(dbg1_sb[:, 192:288], Z1[:, :])
                nc.vector.tensor_copy(dbg1_sb[:, 288:384], rZ1[:, :])
                nc.vector.tensor_copy(dbg1_sb[:, 384:960],
                                      al[:, :, :].rearrange("p t x -> p (t x)"))
                nc.sync.dma_start(out=dbg1[:, :], in_=dbg1_sb[:, :])
                dbg2_sb = work.tile([128, 3072], F32, tag="dbg2sb")
                nc.vector.tensor_copy(dbg2_sb[:, :], attn_f)
                nc.sync.dma_start(out=dbg2[:, :], in_=dbg2_sb[:, :])

            # ---- layer 2: h2 = sum_f v1*W2 (2x mul, 2x fold adds, then a
            # small reduce) ----
            vw = work.tile([128, 24, 128], BF16, tag="vw")
            nc.vector.tensor_tensor(
                vw[:, :, :],
                v1[:, :].rearrange("p (c f) -> p c f", f=128),
                w2_sb[:, :].unsqueeze(1).broadcast_to([128, 24, 128]),
                OP.mult)
            vwa = work.tile([128, 24, 64], BF16, tag="vwa")
            nc.vector.tensor_add(vwa[:, :, :], vw[:, :, 0:64],
                                 vw[:, :, 64:128])
            vwb = work.tile([128, 24, 32], F32, tag="vwb")
            nc.vector.tensor_add(vwb[:, :, :], vwa[:, :, 0:32],
                                 vwa[:, :, 32:64])
            h2 = small.tile([128, 24], F32, tag="h2")
            nc.vector.tensor_reduce(h2[:, :], vwb[:, :, :], axis=AX.X,
                                    op=OP.add)
            h2c = small.tile([128, 24], F32, tag="h2c")
            nc.vector.tensor_scalar(h2c[:, :], h2[:, :], -c2, None, OP.add)
            h2s = small.tile([128, 24], F32, tag="h2s")
            nc.vector.tensor_scalar(h2s[:, :], h2[:, :], as2,
                                    -c2 * (as2 + ad2), OP.mult, OP.add)
            # e2 = ad2*h2[i] + (as2*h2[j] - c2*(as2+ad2)); layout (t, i, j)
            h2d = small.tile([128, 24], F32, tag="h2d")
            nc.vector.tensor_scalar(h2d[:, :], h2[:, :], ad2, None, OP.mult)
            h2dv = h2d[:, :].rearrange("p (t i) -> p t i", t=4)
            h2sv = h2s[:, :].rearrange("p (t j) -> p t j", t=4)
            h2cv = h2c[:, :].rearrange("p (t j) -> p t j", t=4)
            e2_raw = small.tile([128, 144], F32, tag="e2raw")
            nc.vector.tensor_tensor(
                e2_raw[:, :].rearrange("p (t i j) -> p t i j", t=4, j=6),
                h2dv.unsqueeze(3).broadcast_to([128, 4, 6, 6]),
                h2sv.unsqueeze(2).broadcast_to([128, 4, 6, 6]),
                OP.add,
            )
            e2_lr = small.tile([128, 144], F32, tag="e2lr")
            nc.vector.scalar_tensor_tensor(
                e2_lr[:, :], e2_raw[:, :], LRELU_SLOPE, e2_raw[:, :],
                OP.mult, OP.max)
            E2 = small.tile([128, 144], F32, tag="E2")
            nc.scalar.activation(E2[:, :], e2_lr[:, :], ACT.Exp)
            E2v = E2[:, :].rearrange("p (ti j) -> p ti j", j=6)
            Z2 = small.tile([128, 24], F32, tag="Z2")
            nc.vector.tensor_reduce(Z2[:, :], E2v, axis=AX.X, op=OP.add)
            rZ2 = small.tile([128, 24], F32, tag="rZ2")
            nc.vector.reciprocal_approx_fast(rZ2[:, :], Z2[:, :])
            P2 = small.tile([128, 144], F32, tag="P2")
            nc.vector.tensor_tensor(
                P2[:, :].rearrange("p (t i j) -> p t i j", t=4, j=6),
                E2[:, :].rearrange("p (t i j) -> p t i j", t=4, j=6),
                h2cv.unsqueeze(2).broadcast_to([128, 4, 6, 6]),
                OP.mult,
            )
            S2 = small.tile([128, 24], F32, tag="S2")
            nc.vector.tensor_reduce(
                S2[:, :], P2[:, :].rearrange("p (ti j) -> p ti j", j=6),
                axis=AX.X, op=OP.add,
            )
            out_sb = small.tile([128, 24], F32, tag="outsb")
            nc.vector.tensor_mul(out_sb[:, :], S2[:, :], rZ2[:, :])
            if b2 != 0.0:
                nc.vector.tensor_scalar(out_sb[:, :], out_sb[:, :], b2,
                                        None, OP.add)
            nc.sync.dma_start(
                out=bass.AP(tensor=out_ext, offset=0,
                            ap=[[K, 128], [128 * K, NT], [1, K]]),
                in_=out_sb[:, :].rearrange("p (t k) -> p t k", t=NT))

    nc.finalize()
    return nc


def prep_core_inputs(core, org_obs, node_embeddings, substation_embeddings,
                     sub_choice, sub_id_to_elem_id, W_proj, b_proj, W1,
                     a_src1, a_dst1, b1, W2, a_src2, a_dst2, b2):
    """Host-side shard + layout prep for one core (index math and weight
    folding only -- all tensor FLOPs on the batch stay on device)."""
    bf = ml_dtypes.bfloat16
    s = slice(core * BS, (core + 1) * BS)

    obs = np.asarray(org_obs[s], np.float32)
    obs_T = np.zeros((OBS_PAD, BS), np.float32)
    obs_T[:OBS, :] = obs.T
    obs_T4 = obs_T.reshape(4, 128, BS).transpose(1, 0, 2).copy()

    comb = np.concatenate(
        [np.asarray(node_embeddings[s], np.float32),
         np.asarray(substation_embeddings[s], np.float32)], axis=1
    ).reshape(BS * R, H).astype(bf)

    sub_idx = np.asarray(sub_choice[s, 0], np.int64)
    elem = np.asarray(sub_id_to_elem_id, np.int64)[sub_idx]   # [BS, K]

    idx_comb = np.zeros((NT, 128, 56), np.int16)
    for t in range(NT):
        bloc = np.arange(128)
        el = elem[t * 128:(t + 1) * 128]
        Ln = (bloc[None, :] * R + el.T)                        # [K, 128]
        Ls = bloc * R + N + sub_idx[t * 128:(t + 1) * 128]
        L = np.concatenate([Ln[:4].reshape(-1), Ls,
                            Ln[4:].reshape(-1)])               # 896
        blk = L.reshape(56, 16).T.astype(np.int16)
        idx_comb[t] = np.tile(blk, (8, 1))
    idx16 = np.zeros((16, 256), np.int16)
    idx16[:, :NT * 56] = idx_comb.transpose(1, 0, 2).reshape(128, NT * 56)[:16]

    W1 = np.asarray(W1, np.float32)
    W1a, W1b, W1c = W1[0:H], W1[H:2 * H], W1[2 * H:3 * H]
    Wp = np.asarray(W_proj, np.float32)
    wfold = np.zeros((OBS_PAD, H), np.float32)
    wfold[:OBS] = Wp @ W1a
    wfold4 = wfold.reshape(4, 128, H).transpose(1, 0, 2).copy()
    bias_h = np.asarray(b1, np.float32) + np.asarray(b_proj, np.float32) @ W1a

    asrc_m = np.zeros((H, 8), np.float32)
    for h in range(HEADS):
        asrc_m[h * FH:(h + 1) * FH, h] = a_src1[h]
        asrc_m[h * FH:(h + 1) * FH, 4 + h] = a_dst1[h]
    # first 128 cols: permutation (h,f)->(f,h) so the transpose emits the
    # apply-friendly layout directly
    perm = np.zeros((H, H), np.float32)
    for h in range(HEADS):
        for f in range(FH):
            perm[h * FH + f, f * HEADS + h] = 1.0
    trident = np.concatenate([perm, asrc_m], axis=1)

    # e must be computed from h WITHOUT b1 (but WITH the obs-projection
    # bias, which is part of the reference h) -> correct only for b1.
    bh = np.asarray(b1, np.float32).reshape(HEADS, FH)
    cek = np.zeros((K, 8), np.float32)
    cek[:, 0:4] = (bh * np.asarray(a_src1, np.float32)).sum(-1)[None, :]
    cek[:, 4:8] = (bh * np.asarray(a_dst1, np.float32)).sum(-1)[None, :]

    return {
        "obs_T": obs_T4.astype(bf),
        "comb_emb": comb,
        "idx16": idx16,
        "wfold": wfold4.astype(bf),
        "w1bc": np.stack([W1b, W1c], axis=1).astype(bf),
        "trident": trident.astype(bf),
        "biash": bias_h.reshape(H, 1).astype(np.float32),
        "ce": cek.reshape(1, 48).astype(np.float32),
        # W2 permuted to the (f,h) feature order used by the apply layout
        "w2r": np.asarray(W2, np.float32).reshape(HEADS, FH).T
               .reshape(1, H).astype(bf),
        "identd": np.eye(H, dtype=np.float32).astype(bf),
    }


_GRAPH_CACHE = {}
LAST_RESULTS = None


def kernel(**inputs):
    inp = {k: np.asarray(v) for k, v in inputs.items()}
    W2 = np.asarray(inp["W2"], np.float32)
    scalars = {
        "a_src2": float(np.asarray(inp["a_src2"]).reshape(-1)[0]),
        "a_dst2": float(np.asarray(inp["a_dst2"]).reshape(-1)[0]),
        "b2": float(np.asarray(inp["b2"]).reshape(-1)[0]),
        "c2": float(W2.sum()),
    }
    key = tuple(sorted(scalars.items()))
    if key not in _GRAPH_CACHE:
        _GRAPH_CACHE[key] = build_graph(scalars)
    nc = _GRAPH_CACHE[key]

    in_maps = [
        prep_core_inputs(
            c, inp["org_obs"], inp["node_embeddings"],
            inp["substation_embeddings"], inp["sub_choice"],
            inp["sub_id_to_elem_id"], inp["W_proj"], inp["b_proj"], inp["W1"],
            inp["a_src1"], inp["a_dst1"], inp["b1"], inp["W2"], inp["a_src2"],
            inp["a_dst2"], inp["b2"],
        )
        for c in range(NCORES)
    ]
    res = run_bass_kernel_spmd(nc, in_maps, core_ids=list(range(NCORES)))
    global LAST_RESULTS
    LAST_RESULTS = res
    out = np.concatenate([res.results[c]["out"] for c in range(NCORES)], axis=0)
    return out.reshape(B, K, 1).astype(np.float32)


if __name__ == "__main__":
    g = build_graph({"a_src2": 0.01, "a_dst2": 0.02, "b2": 0.0, "c2": 0.1})
    print("graph built ok")


# revision 42
# speedup vs baseline: 1.0381x; 1.0381x over previous
"""Trainium2 Bass kernel for nn_Action_Decoder (GAT-based action decoder).

v2 strategy (8 NeuronCores, pure data-parallel over batch):
  - B=4096 sharded 8 x 512 samples/core; weights replicated; 4 tiles of
    128 samples on the partition dim.
  - Host folds W_proj@W1_obs into one [512,128] block (weight folding
    only), so the obs projection feeds GAT layer 1 directly.
  - Gathers: one combined [node|sub] bf16 table per sample in DRAM; two
    dma_gather(transpose=True) chunks per tile on 8 SWDGE queues, issued
    up-front.
  - Layer-1: h^T = W^T x^T via PE with stationary reuse; the obs+sub
    "shared" part is computed once per tile ([128,128]) and broadcast
    over the 6 nodes with an identity-stationary matmul (0-stride
    moving operand).
  - Fused transpose+e: per node k, ONE matmul with stationary hT_k and
    moving [ident(128) | a_src/a_dst(8)] yields both the batch-layout
    h block and the e_src/e_dst values.
  - All per-sample phases (softmax, apply, elu, layer 2) run batched
    across the 4 tiles to amortize per-instruction overhead; alpha is
    stored (t,i,j,h) so the apply multiplies use a 2D access pattern.
  - elu (exact: exp(min(x,0)) + relu(x) - 1, -1 folded into sum(W2)) +
    layer-2 GAT via affine_mul_reduce + a batched 6x6 attention.
"""

import os
import sys

import numpy as np

for _p in ("/root/.axon_site", "/root/.axon_site/_ro/trn_rl_repo",
           "/root/.axon_site/_ro/pypackages", "/opt/trn_rl_repo", "/opt/pypackages"):
    if os.path.isdir(_p) and _p not in sys.path:
        sys.path.append(_p)

import ml_dtypes

import concourse.bass as bass
import concourse.tile as tile
from concourse import bacc
from concourse import mybir
from concourse.bass_utils import run_bass_kernel_spmd

# Problem dims
B, N, S, K, H, OBS = 4096, 177, 36, 6, 128, 500
HEADS, FH = 4, 32
NCORES = 8
BS = B // NCORES          # 512 samples per core
NT = BS // 128            # 4 tiles of 128 samples
OBS_PAD = 512             # pad 500 -> 512
R = N + S                 # combined table rows per sample (213)

F32 = mybir.dt.float32
BF16 = mybir.dt.bfloat16
I16 = mybir.dt.int16
AX = mybir.AxisListType
OP = mybir.AluOpType
ACT = mybir.ActivationFunctionType

LRELU_SLOPE = 0.2
FOLDS_ON_GPSIMD = False
GPSIMD_MULS = True
DEBUG_DUMP = False


def build_graph(scalars):
    as2 = float(scalars["a_src2"])
    ad2 = float(scalars["a_dst2"])
    b2 = float(scalars["b2"])
    c2 = float(scalars["c2"])

    nc = bacc.Bacc(num_swdge_queues=4)

    obs_T = nc.declare_dram_parameter("obs_T", [4, 128, 512], BF16, isOutput=False)
    comb_emb = nc.declare_dram_parameter("comb_emb", [BS * R, H], BF16, isOutput=False)
    idx16 = nc.declare_dram_parameter("idx16", [16, 256], I16, isOutput=False)
    wfold = nc.declare_dram_parameter("wfold", [128, 4 * H], BF16, isOutput=False)
    w1bc = nc.declare_dram_parameter("w1bc", [H, 2, H], BF16, isOutput=False)
    trident = nc.declare_dram_parameter("trident", [H, 136], BF16, isOutput=False)
    identd = nc.declare_dram_parameter("identd", [H, H], BF16, isOutput=False)
    biash = nc.declare_dram_parameter("biash", [H, 1], F32, isOutput=False)
    ce = nc.declare_dram_parameter("ce", [1, 48], F32, isOutput=False)
    w2r = nc.declare_dram_parameter("w2r", [1, H], BF16, isOutput=False)
    out_ext = nc.declare_dram_parameter("out", [BS, K], F32, isOutput=True)
    if DEBUG_DUMP:
        dbg1 = nc.declare_dram_parameter("dbg1", [128, 960], F32, isOutput=True)
        dbg2 = nc.declare_dram_parameter("dbg2", [128, 3072], F32, isOutput=True)

    with tile.TileContext(nc) as tc:
        with (
            tc.tile_pool(name="consts", bufs=1) as consts,
            tc.tile_pool(name="gat", bufs=4) as gat,
            tc.tile_pool(name="ht", bufs=2) as htp,
            tc.tile_pool(name="work", bufs=1) as work,
            tc.tile_pool(name="small", bufs=3) as small,
            tc.tile_pool(name="psH", bufs=2, space="PSUM") as psH,
            tc.tile_pool(name="psS", bufs=1, space="PSUM") as psS,
            tc.tile_pool(name="psT", bufs=1, space="PSUM") as psT,
        ):
            # ---- idx bootstrap: the static DMA rings take ~13us to fill
            # a [128,*] tile (round-robin with the big const DMAs), so load
            # the idx table via a SWDGE gather instead. The channel-index
            # tile (p%16) is built with 16 partition-strided memsets. ----
            chp = consts.tile([128, 8], I16)
            nc.gpsimd.iota(chp[:, :], pattern=[[0, 8]], base=0,
                           channel_multiplier=1)
            ch16 = consts.tile([128, 8], I16)
            nc.vector.tensor_scalar(ch16[:, :], chp[:, :], 15, None,
                                    OP.bitwise_and)
            iot16 = consts.tile([128, 8], I16)
            nc.gpsimd.iota(iot16[:, :], pattern=[[16, 8]], base=0,
                           channel_multiplier=0)
            idfull = consts.tile([128, 8], I16)
            nc.vector.tensor_add(idfull[:, :], ch16[:, :], iot16[:, :])
            idx_sb = consts.tile([128, 256], I16)
            nc.gpsimd.dma_gather(
                out_ap=idx_sb[:, :].unsqueeze(1),
                in_ap=idx16[:, :],
                idxs_ap=ch16[:, :],
                num_idxs=128, num_idxs_reg=128, elem_size=256,
                transpose=False, queue_num=0,
            )

            # ---- big consts via SWDGE identity-gathers (static rings
            # would contend with the gpsimd library load) ----
            wfold_sb = consts.tile([128, 4, 128], BF16)
            nc.gpsimd.dma_gather(
                out_ap=wfold_sb[:, :, :].rearrange("p c f -> p (c f)")
                .unsqueeze(1),
                in_ap=wfold[:, :], idxs_ap=idfull[:, :],
                num_idxs=128, num_idxs_reg=128, elem_size=4 * H,
                transpose=False, queue_num=1,
            )
            w1bc_sb = consts.tile([128, 2, 128], BF16)
            nc.scalar.dma_start(out=w1bc_sb[:, :, :], in_=w1bc[:, :, :])
            tri_sb = consts.tile([128, 136], BF16)
            nc.scalar.dma_start(out=tri_sb, in_=trident[:, :])
            ident_sb = consts.tile([128, 128], BF16)
            nc.scalar.dma_start(out=ident_sb, in_=identd[:, :])
            biash_sb = consts.tile([128, 1], F32)
            nc.scalar.dma_start(out=biash_sb, in_=biash[:, :])
            ce_sb = consts.tile([128, 48], F32)
            nc.scalar.dma_start(
                out=ce_sb,
                in_=bass.AP(tensor=ce, offset=0, ap=[[0, 128], [1, 48]]),
            )
            w2_sb = consts.tile([128, 128], BF16)
            nc.scalar.dma_start(
                out=w2_sb,
                in_=bass.AP(tensor=w2r, offset=0, ap=[[0, 128], [1, 128]]),
            )
            obs_sb = consts.tile([128, 4, 4, 128], BF16)
            for t in range(NT):
                nc.gpsimd.dma_gather(
                    out_ap=obs_sb[:, t, :, :].rearrange("p c f -> p (c f)")
                    .unsqueeze(1),
                    in_ap=obs_T[t, :, :], idxs_ap=idfull[:, :],
                    num_idxs=128, num_idxs_reg=128, elem_size=512,
                    transpose=False, queue_num=t,
                )

            # ---- gathers: 2 chunks per tile, 8 queues, issued up-front ----
            gth = []
            for t in range(NT):
                g = gat.tile([128, 896], BF16, tag=f"g{t}")
                nc.gpsimd.dma_gather(
                    out_ap=g[:, 0:640].rearrange("p (x n) -> p x n", x=1),
                    in_ap=comb_emb[t * 128 * R:(t + 1) * 128 * R, :],
                    idxs_ap=idx_sb[:, t * 56:t * 56 + 40],
                    num_idxs=640, num_idxs_reg=640, elem_size=H,
                    transpose=True, queue_num=(t % 2) * 2 + (t // 2),
                )
                nc.gpsimd.dma_gather(
                    out_ap=g[:, 640:896].rearrange("p (x n) -> p x n", x=1),
                    in_ap=comb_emb[t * 128 * R:(t + 1) * 128 * R, :],
                    idxs_ap=idx_sb[:, t * 56 + 40:t * 56 + 56],
                    num_idxs=256, num_idxs_reg=256, elem_size=H,
                    transpose=True, queue_num=(t % 2) * 2 + 1 - (t // 2),
                )
                gth.append(g)

            # ---- per-tile: shared (obs+sub), node matmuls, broadcast-add,
            # transposes. NOTE: each sh region's 5 accumulating matmuls must
            # be consecutive -- a start=True on the shared PSUM bank clears
            # has_written bits for the WHOLE bank. ----
            hbL = work.tile([128, NT, 768], BF16, tag="hbL")
            e_sd = small.tile([128, NT, 48], F32, tag="esd")
            sh_sb = consts.tile([128, 4, 128], BF16)
            sh_ps = psS.tile([128, 4, 128], F32)
            for t in range(NT):
                g = gth[t]
                # shared = Wfold^T obs^T + W1b^T sub^T (consecutive group)
                for c in range(4):
                    nc.tensor.matmul(
                        sh_ps[:, t, :], wfold_sb[:, c, :],
                        obs_sb[:, t, c, :],
                        start=(c == 0), stop=False,
                    )
                nc.tensor.matmul(sh_ps[:, t, :], w1bc_sb[:, 0, :],
                                 g[:, 512:640], start=False, stop=True)
                nc.scalar.activation(sh_sb[:, t, :], sh_ps[:, t, :], ACT.Copy)
                h_ps = psH.tile([128, 768], F32, tag="hps")
                # node part (W1c stationary)
                nc.tensor.matmul(h_ps[:, 0:512], w1bc_sb[:, 1, :], g[:, 0:512],
                                 start=True, stop=False)
                nc.tensor.matmul(h_ps[:, 512:768], w1bc_sb[:, 1, :],
                                 g[:, 640:896], start=True, stop=False)
                # broadcast shared over the 6 nodes (ident stationary,
                # 0-stride moving)
                for hs, nk in ((slice(0, 512), 4), (slice(512, 768), 2)):
                    mov = (sh_sb[:, t, :].unsqueeze(1)
                           .broadcast_to([128, nk, 128]))
                    nc.tensor.matmul(h_ps[:, hs], ident_sb[:, :], mov,
                                     start=False, stop=True)
                # h^T (+bias) -> SBUF bf16
                hT = htp.tile([128, 768], BF16, tag="hT")
                nc.scalar.activation(hT[:, :], h_ps[:, :], ACT.Identity,
                                     bias=biash_sb[:, :], scale=1.0)
                # fused transpose + e: out[:,k,0:128]=hbL block,
                # out[:,k,128:136]=e_src/e_dst. Split 3+3 so no matmul
                # output straddles a PSUM bank (136 f32 x 3 fits in 2KB).
                ps2a = psT.tile([128, 3, 136], F32, tag="ps2a")
                ps2b = psT.tile([128, 3, 136], F32, tag="ps2b")
                for k in range(K):
                    ps2k = ps2a[:, k, :] if k < 3 else ps2b[:, k - 3, :]
                    nc.tensor.matmul(
                        ps2k, hT[:, k * 128:(k + 1) * 128],
                        tri_sb[:, :], start=True, stop=True,
                    )
                # trident's perm already emits (f,h) order: plain evicts
                nc.scalar.activation(
                    hbL[:, t, 0:384].rearrange("p (k f) -> p k f", k=3),
                    ps2a[:, :, 0:128], ACT.Copy)
                nc.scalar.activation(
                    hbL[:, t, 384:768].rearrange("p (k f) -> p k f", k=3),
                    ps2b[:, :, 0:128], ACT.Copy)
                nc.vector.tensor_sub(
                    e_sd[:, t, 0:24].rearrange("p (k s) -> p k s", k=3),
                    ps2a[:, :, 128:136], ce_sb[:, 0:24].rearrange(
                        "p (k s) -> p k s", k=3))
                nc.vector.tensor_sub(
                    e_sd[:, t, 24:48].rearrange("p (k s) -> p k s", k=3),
                    ps2b[:, :, 128:136], ce_sb[:, 24:48].rearrange(
                        "p (k s) -> p k s", k=3))

            # ---- alpha + apply, pipelined over tile PAIRS so the vector
            # phase starts as soon as tiles 0/1 land. Layout (t,i,j,h);
            # apply muls in (j,f,h) (h-innermost step-1 -> DVE 2x);
            # three batched fold adds per half. ----
            e4 = e_sd[:, :, :].rearrange("p t (k s h) -> p t k s h", s=2, h=4)
            e_raw = small.tile([128, NT, 144], BF16, tag="eraw")
            e_lrf = small.tile([128, NT * 144], BF16, tag="elr")
            E1 = small.tile([128, NT * 144], F32, tag="E1")
            Z1 = small.tile([128, 96], F32, tag="Z1")
            rZ1 = small.tile([128, 96], F32, tag="rZ1")
            al = small.tile([128, NT, 144], BF16, tag="al")
            prod = work.tile([128, 24, 768], BF16, tag="prod")
            f1 = work.tile([128, 24, 384], BF16, tag="f1")
            f2 = work.tile([128, 24, 128], BF16, tag="f2")
            attn = work.tile([128, 24, 128], BF16, tag="attn")
            e_rawf = e_raw[:, :, :].rearrange("p t x -> p (t x)")
            for half in range(2):
                th0 = 2 * half
                for t in (th0, th0 + 1):
                    e_dst_ap = (e4[:, t, :, 1, :].unsqueeze(2)
                                .broadcast_to([128, 6, 6, 4]))
                    e_src_ap = (e4[:, t, :, 0, :].unsqueeze(1)
                                .broadcast_to([128, 6, 6, 4]))
                    nc.vector.tensor_tensor(
                        e_raw[:, t, :].rearrange("p (i j h) -> p i j h",
                                                 j=6, h=4),
                        e_dst_ap, e_src_ap, OP.add,
                    )
                sl = slice(th0 * 144, (th0 + 2) * 144)
                zs = slice(half * 48, (half + 1) * 48)
                nc.vector.scalar_tensor_tensor(
                    e_lrf[:, sl], e_rawf[:, sl], LRELU_SLOPE, e_rawf[:, sl],
                    OP.mult, OP.max)
                nc.scalar.activation(E1[:, sl], e_lrf[:, sl], ACT.Exp)
                nc.vector.tensor_reduce(
                    Z1[:, zs],
                    E1[:, sl].rearrange("p (ti j h) -> p ti j h", j=6, h=4)
                    .transpose([0, 1, 3, 2]),
                    axis=AX.X, op=OP.add)
                nc.vector.reciprocal_approx_fast(rZ1[:, zs], Z1[:, zs])
                nc.vector.tensor_tensor(
                    al[:, th0:th0 + 2, :].rearrange(
                        "p t (i j h) -> p (t i) j h", j=6, h=4),
                    E1[:, sl].rearrange("p (ti j h) -> p ti j h", j=6, h=4),
                    rZ1[:, zs].rearrange("p (ti h) -> p ti h", h=4)
                    .unsqueeze(2).broadcast_to([128, 12, 6, 4]),
                    OP.mult,
                )
                for i in range(K):
                    eng = nc.gpsimd if (GPSIMD_MULS and i >= 4) else nc.vector
                    for t in (th0, th0 + 1):
                        eng.tensor_tensor(
                            prod[:, t * 6 + i, :].rearrange(
                                "p (j f h) -> p j f h", f=32, h=4),
                            al[:, t, i * 24:(i + 1) * 24]
                            .rearrange("p (j h) -> p j h", h=4)
                            .unsqueeze(2).broadcast_to([128, 6, 32, 4]),
                            hbL[:, t, :].rearrange("p (j f h) -> p j f h",
                                                   f=32, h=4),
                            OP.mult,
                        )
                cs = slice(th0 * 6, (th0 + 2) * 6)
                nc.vector.tensor_add(f1[:, cs, :], prod[:, cs, 0:384],
                                     prod[:, cs, 384:768])
                nc.vector.tensor_add(f2[:, cs, :], f1[:, cs, 0:128],
                                     f1[:, cs, 128:256])
                nc.vector.tensor_add(attn[:, cs, :], f2[:, cs, :],
                                     f1[:, cs, 256:384])

            # ---- elu(x) = exp(min(x,0)) + relu(x) - 1 (-1 folded into c2)
            attn_f = attn[:, :, :].rearrange("p c f -> p (c f)")
            min_x = work.tile([128, NT * 768], BF16, tag="minx")
            exp_m = work.tile([128, NT * 768], BF16, tag="expm")
            relu_x = work.tile([128, NT * 768], BF16, tag="relux")
            v1 = work.tile([128, NT * 768], BF16, tag="v1")
            HB = NT * 384
            for hh in range(2):
                es = slice(hh * HB, (hh + 1) * HB)
                nc.vector.tensor_scalar_min(min_x[:, es], attn_f[:, es], 0.0)
                nc.scalar.activation(exp_m[:, es], min_x[:, es], ACT.Exp)
                nc.vector.tensor_scalar_max(relu_x[:, es], attn_f[:, es], 0.0)
                nc.vector.tensor_add(v1[:, es], exp_m[:, es], relu_x[:, es])

            if DEBUG_DUMP:
                # dbg1: e_sd (192) | Z1 (96) | rZ1 (96) | al (576, as f32)
                dbg1_sb = work.tile([128, 960], F32, tag="dbg1sb")
                nc.vector.tensor_copy(dbg1_sb[:, 0:192],
                                      e_sd[:, :, :].rearrange("p t x -> p (t x)"))
                nc.vector.tensor_copy(dbg1_sb[:, 192:288], Z1[:, :])
                nc.vector.tensor_copy(dbg1_sb[:, 288:384], rZ1[:, :])
                nc.vector.tensor_copy(dbg1_sb[:, 384:960],
                                      al[:, :, :].rearrange("p t x -> p (t x)"))
                nc.sync.dma_start(out=dbg1[:, :], in_=dbg1_sb[:, :])
                dbg2_sb = work.tile([128, 3072], F32, tag="dbg2sb")
                nc.vector.tensor_copy(dbg2_sb[:, :], attn_f)
                nc.sync.dma_start(out=dbg2[:, :], in_=dbg2_sb[:, :])

            # ---- layer 2: h2 = sum_f v1*W2 (2x mul, 2x fold adds, then a
            # small reduce) ----
            vw = work.tile([128, 24, 128], BF16, tag="vw")
            nc.vector.tensor_tensor(
                vw[:, :, :],
                v1[:, :].rearrange("p (c f) -> p c f", f=128),
                w2_sb[:, :].unsqueeze(1).broadcast_to([128, 24, 128]),
                OP.mult)
            vwa = work.tile([128, 24, 64], BF16, tag="vwa")
            nc.vector.tensor_add(vwa[:, :, :], vw[:, :, 0:64],
                                 vw[:, :, 64:128])
            vwb = work.tile([128, 24, 32], F32, tag="vwb")
            nc.vector.tensor_add(vwb[:, :, :], vwa[:, :, 0:32],
                                 vwa[:, :, 32:64])
            h2 = small.tile([128, 24], F32, tag="h2")
            nc.vector.tensor_reduce(h2[:, :], vwb[:, :, :], axis=AX.X,
                                    op=OP.add)
            h2c = small.tile([128, 24], F32, tag="h2c")
            nc.vector.tensor_scalar(h2c[:, :], h2[:, :], -c2, None, OP.add)
            h2s = small.tile([128, 24], F32, tag="h2s")
            nc.vector.tensor_scalar(h2s[:, :], h2[:, :], as2,
                                    -c2 * (as2 + ad2), OP.mult, OP.add)
            # e2 = ad2*h2[i] + (as2*h2[j] - c2*(as2+ad2)); layout (t, i, j)
            h2d = small.tile([128, 24], F32, tag="h2d")
            nc.vector.tensor_scalar(h2d[:, :], h2[:, :], ad2, None, OP.mult)
            h2dv = h2d[:, :].rearrange("p (t i) -> p t i", t=4)
            h2sv = h2s[:, :].rearrange("p (t j) -> p t j", t=4)
            h2cv = h2c[:, :].rearrange("p (t j) -> p t j", t=4)
            e2_raw = small.tile([128, 144], F32, tag="e2raw")
            nc.vector.tensor_tensor(
                e2_raw[:, :].rearrange("p (t i j) -> p t i j", t=4, j=6),
                h2dv.unsqueeze(3).broadcast_to([128, 4, 6, 6]),
                h2sv.unsqueeze(2).broadcast_to([128, 4, 6, 6]),
                OP.add,
            )
            e2_lr = small.tile([128, 144], F32, tag="e2lr")
            nc.vector.scalar_tensor_tensor(
                e2_lr[:, :], e2_raw[:, :], LRELU_SLOPE, e2_raw[:, :],
                OP.mult, OP.max)
            E2 = small.tile([128, 144], F32, tag="E2")
            nc.scalar.activation(E2[:, :], e2_lr[:, :], ACT.Exp)
            E2v = E2[:, :].rearrange("p (ti j) -> p ti j", j=6)
            Z2 = small.tile([128, 24], F32, tag="Z2")
            nc.vector.tensor_reduce(Z2[:, :], E2v, axis=AX.X, op=OP.add)
            rZ2 = small.tile([128, 24], F32, tag="rZ2")
            nc.vector.reciprocal_approx_fast(rZ2[:, :], Z2[:, :])
            P2 = small.tile([128, 144], F32, tag="P2")
            nc.vector.tensor_tensor(
                P2[:, :].rearrange("p (t i j) -> p t i j", t=4, j=6),
                E2[:, :].rearrange("p (t i j) -> p t i j", t=4, j=6),
                h2cv.unsqueeze(2).broadcast_to([128, 4, 6, 6]),
                OP.mult,
            )
            S2 = small.tile([128, 24], F32, tag="S2")
            nc.vector.tensor_reduce(
                S2[:, :], P2[:, :].rearrange("p (ti j) -> p ti j", j=6),
                axis=AX.X, op=OP.add,
            )
            out_sb = small.tile([128, 24], F32, tag="outsb")
            nc.vector.tensor_mul(out_sb[:, :], S2[:, :], rZ2[:, :])
            if b2 != 0.0:
                nc.vector.tensor_scalar(out_sb[:, :], out_sb[:, :], b2,
                                        None, OP.add)
            nc.sync.dma_start(
                out=bass.AP(tensor=out_ext, offset=0,
                            ap=[[K, 128], [128 * K, NT], [1, K]]),
                in_=out_sb[:, :].rearrange("p (t k) -> p t k", t=NT))

    nc.finalize()
    return nc


def prep_core_inputs(core, org_obs, node_embeddings, substation_embeddings,
                     sub_choice, sub_id_to_elem_id, W_proj, b_proj, W1,
                     a_src1, a_dst1, b1, W2, a_src2, a_dst2, b2):
    """Host-side shard + layout prep for one core (index math and weight
    folding only -- all tensor FLOPs on the batch stay on device)."""
    bf = ml_dtypes.bfloat16
    s = slice(core * BS, (core + 1) * BS)

    obs = np.asarray(org_obs[s], np.float32)
    obs_T = np.zeros((OBS_PAD, BS), np.float32)
    obs_T[:OBS, :] = obs.T
    obs_T4 = obs_T.reshape(4, 128, NT, 128).transpose(2, 1, 0, 3).reshape(
        NT, 128, 512).copy()

    comb = np.concatenate(
        [np.asarray(node_embeddings[s], np.float32),
         np.asarray(substation_embeddings[s], np.float32)], axis=1
    ).reshape(BS * R, H).astype(bf)

    sub_idx = np.asarray(sub_choice[s, 0], np.int64)
    elem = np.asarray(sub_id_to_elem_id, np.int64)[sub_idx]   # [BS, K]

    idx_comb = np.zeros((NT, 128, 56), np.int16)
    for t in range(NT):
        bloc = np.arange(128)
        el = elem[t * 128:(t + 1) * 128]
        Ln = (bloc[None, :] * R + el.T)                        # [K, 128]
        Ls = bloc * R + N + sub_idx[t * 128:(t + 1) * 128]
        L = np.concatenate([Ln[:4].reshape(-1), Ls,
                            Ln[4:].reshape(-1)])               # 896
        blk = L.reshape(56, 16).T.astype(np.int16)
        idx_comb[t] = np.tile(blk, (8, 1))
    idx16 = np.zeros((16, 256), np.int16)
    idx16[:, :NT * 56] = idx_comb.transpose(1, 0, 2).reshape(128, NT * 56)[:16]

    W1 = np.asarray(W1, np.float32)
    W1a, W1b, W1c = W1[0:H], W1[H:2 * H], W1[2 * H:3 * H]
    Wp = np.asarray(W_proj, np.float32)
    wfold = np.zeros((OBS_PAD, H), np.float32)
    wfold[:OBS] = Wp @ W1a
    wfold4 = wfold.reshape(4, 128, H).transpose(1, 0, 2).reshape(
        128, 4 * H).copy()
    bias_h = np.asarray(b1, np.float32) + np.asarray(b_proj, np.float32) @ W1a

    asrc_m = np.zeros((H, 8), np.float32)
    for h in range(HEADS):
        asrc_m[h * FH:(h + 1) * FH, h] = a_src1[h]
        asrc_m[h * FH:(h + 1) * FH, 4 + h] = a_dst1[h]
    # first 128 cols: permutation (h,f)->(f,h) so the transpose emits the
    # apply-friendly layout directly
    perm = np.zeros((H, H), np.float32)
    for h in range(HEADS):
        for f in range(FH):
            perm[h * FH + f, f * HEADS + h] = 1.0
    trident = np.concatenate([perm, asrc_m], axis=1)

    # e must be computed from h WITHOUT b1 (but WITH the obs-projection
    # bias, which is part of the reference h) -> correct only for b1.
    bh = np.asarray(b1, np.float32).reshape(HEADS, FH)
    cek = np.zeros((K, 8), np.float32)
    cek[:, 0:4] = (bh * np.asarray(a_src1, np.float32)).sum(-1)[None, :]
    cek[:, 4:8] = (bh * np.asarray(a_dst1, np.float32)).sum(-1)[None, :]

    return {
        "obs_T": obs_T4.astype(bf),
        "comb_emb": comb,
        "idx16": idx16,
        "wfold": wfold4.astype(bf),
        "w1bc": np.stack([W1b, W1c], axis=1).astype(bf),
        "trident": trident.astype(bf),
        "biash": bias_h.reshape(H, 1).astype(np.float32),
        "ce": cek.reshape(1, 48).astype(np.float32),
        # W2 permuted to the (f,h) feature order used by the apply layout
        "w2r": np.asarray(W2, np.float32).reshape(HEADS, FH).T
               .reshape(1, H).astype(bf),
        "identd": np.eye(H, dtype=np.float32).astype(bf),
    }


_GRAPH_CACHE = {}
LAST_RESULTS = None


def kernel(**inputs):
    inp = {k: np.asarray(v) for k, v in inputs.items()}
    W2 = np.asarray(inp["W2"], np.float32)
    scalars = {
        "a_src2": float(np.asarray(inp["a_src2"]).reshape(-1)[0]),
        "a_dst2": float(np.asarray(inp["a_dst2"]).reshape(-1)[0]),
        "b2": float(np.asarray(inp["b2"]).reshape(-1)[0]),
        "c2": float(W2.sum()),
    }
    key = tuple(sorted(scalars.items()))
    if key not in _GRAPH_CACHE:
        _GRAPH_CACHE[key] = build_graph(scalars)
    nc = _GRAPH_CACHE[key]

    in_maps = [
        prep_core_inputs(
            c, inp["org_obs"], inp["node_embeddings"],
            inp["substation_embeddings"], inp["sub_choice"],
            inp["sub_id_to_elem_id"], inp["W_proj"], inp["b_proj"], inp["W1"],
            inp["a_src1"], inp["a_dst1"], inp["b1"], inp["W2"], inp["a_src2"],
            inp["a_dst2"], inp["b2"],
        )
        for c in range(NCORES)
    ]
    res = run_bass_kernel_spmd(nc, in_maps, core_ids=list(range(NCORES)))
    global LAST_RESULTS
    LAST_RESULTS = res
    out = np.concatenate([res.results[c]["out"] for c in range(NCORES)], axis=0)
    return out.reshape(B, K, 1).astype(np.float32)


if __name__ == "__main__":
    g = build_graph({"a_src2": 0.01, "a_dst2": 0.02, "b2": 0.0, "c2": 0.1})
    print("graph built ok")


# revision 43
# speedup vs baseline: 1.0556x; 1.0168x over previous
"""Trainium2 Bass kernel for nn_Action_Decoder (GAT-based action decoder).

v2 strategy (8 NeuronCores, pure data-parallel over batch):
  - B=4096 sharded 8 x 512 samples/core; weights replicated; 4 tiles of
    128 samples on the partition dim.
  - Host folds W_proj@W1_obs into one [512,128] block (weight folding
    only), so the obs projection feeds GAT layer 1 directly.
  - Gathers: one combined [node|sub] bf16 table per sample in DRAM; two
    dma_gather(transpose=True) chunks per tile on 8 SWDGE queues, issued
    up-front.
  - Layer-1: h^T = W^T x^T via PE with stationary reuse; the obs+sub
    "shared" part is computed once per tile ([128,128]) and broadcast
    over the 6 nodes with an identity-stationary matmul (0-stride
    moving operand).
  - Fused transpose+e: per node k, ONE matmul with stationary hT_k and
    moving [ident(128) | a_src/a_dst(8)] yields both the batch-layout
    h block and the e_src/e_dst values.
  - All per-sample phases (softmax, apply, elu, layer 2) run batched
    across the 4 tiles to amortize per-instruction overhead; alpha is
    stored (t,i,j,h) so the apply multiplies use a 2D access pattern.
  - elu (exact: exp(min(x,0)) + relu(x) - 1, -1 folded into sum(W2)) +
    layer-2 GAT via affine_mul_reduce + a batched 6x6 attention.
"""

import os
import sys

import numpy as np

for _p in ("/root/.axon_site", "/root/.axon_site/_ro/trn_rl_repo",
           "/root/.axon_site/_ro/pypackages", "/opt/trn_rl_repo", "/opt/pypackages"):
    if os.path.isdir(_p) and _p not in sys.path:
        sys.path.append(_p)

import ml_dtypes

import concourse.bass as bass
import concourse.tile as tile
from concourse import bacc
from concourse import mybir
from concourse.bass_utils import run_bass_kernel_spmd

# Problem dims
B, N, S, K, H, OBS = 4096, 177, 36, 6, 128, 500
HEADS, FH = 4, 32
NCORES = 8
BS = B // NCORES          # 512 samples per core
NT = BS // 128            # 4 tiles of 128 samples
OBS_PAD = 512             # pad 500 -> 512
R = N + S                 # combined table rows per sample (213)

F32 = mybir.dt.float32
BF16 = mybir.dt.bfloat16
I16 = mybir.dt.int16
AX = mybir.AxisListType
OP = mybir.AluOpType
ACT = mybir.ActivationFunctionType

LRELU_SLOPE = 0.2
FOLDS_ON_GPSIMD = False
GPSIMD_MULS = True
DEBUG_DUMP = False


def build_graph(scalars):
    as2 = float(scalars["a_src2"])
    ad2 = float(scalars["a_dst2"])
    b2 = float(scalars["b2"])
    c2 = float(scalars["c2"])

    nc = bacc.Bacc(num_swdge_queues=4)

    obs_T = nc.declare_dram_parameter("obs_T", [128, 4, BS], BF16, isOutput=False)
    comb_emb = nc.declare_dram_parameter("comb_emb", [BS * R, H], BF16, isOutput=False)
    idx_comb = nc.declare_dram_parameter("idx_comb", [128, NT * 56], I16, isOutput=False)
    wfold = nc.declare_dram_parameter("wfold", [128, 4, H], BF16, isOutput=False)
    w1bc = nc.declare_dram_parameter("w1bc", [H, 2, H], BF16, isOutput=False)
    trident = nc.declare_dram_parameter("trident", [H, 136], BF16, isOutput=False)
    identd = nc.declare_dram_parameter("identd", [H, H], BF16, isOutput=False)
    biash = nc.declare_dram_parameter("biash", [H, 1], F32, isOutput=False)
    ce = nc.declare_dram_parameter("ce", [1, 48], F32, isOutput=False)
    w2r = nc.declare_dram_parameter("w2r", [1, H], BF16, isOutput=False)
    out_ext = nc.declare_dram_parameter("out", [BS, K], F32, isOutput=True)
    if DEBUG_DUMP:
        dbg1 = nc.declare_dram_parameter("dbg1", [128, 960], F32, isOutput=True)
        dbg2 = nc.declare_dram_parameter("dbg2", [128, 3072], F32, isOutput=True)

    with tile.TileContext(nc) as tc:
        with (
            tc.tile_pool(name="consts", bufs=1) as consts,
            tc.tile_pool(name="gat", bufs=4) as gat,
            tc.tile_pool(name="ht", bufs=2) as htp,
            tc.tile_pool(name="work", bufs=1) as work,
            tc.tile_pool(name="small", bufs=3) as small,
            tc.tile_pool(name="psH", bufs=2, space="PSUM") as psH,
            tc.tile_pool(name="psS", bufs=1, space="PSUM") as psS,
            tc.tile_pool(name="psT", bufs=1, space="PSUM") as psT,
        ):
            # ---- idx tile first, from the gpsimd engine so the gathers
            # depend only on it (not on unrelated const DMA sems) ----
            idx_sb = consts.tile([128, NT, 56], I16)
            nc.scalar.dma_start(
                out=idx_sb[:, :, :].rearrange("p t c -> p (t c)"),
                in_=idx_comb[:, :],
            )

            # ---- constants on other queues ----
            wfold_sb = consts.tile([128, 4, 128], BF16)
            nc.sync.dma_start(out=wfold_sb[:, :, :], in_=wfold[:, :, :])
            w1bc_sb = consts.tile([128, 2, 128], BF16)
            nc.scalar.dma_start(out=w1bc_sb[:, :, :], in_=w1bc[:, :, :])
            tri_sb = consts.tile([128, 136], BF16)
            nc.scalar.dma_start(out=tri_sb, in_=trident[:, :])
            ident_sb = consts.tile([128, 128], BF16)
            nc.scalar.dma_start(out=ident_sb, in_=identd[:, :])
            biash_sb = consts.tile([128, 1], F32)
            nc.scalar.dma_start(out=biash_sb, in_=biash[:, :])
            ce_sb = consts.tile([128, 48], F32)
            nc.scalar.dma_start(
                out=ce_sb,
                in_=bass.AP(tensor=ce, offset=0, ap=[[0, 128], [1, 48]]),
            )
            w2_sb = consts.tile([128, 128], BF16)
            nc.scalar.dma_start(
                out=w2_sb,
                in_=bass.AP(tensor=w2r, offset=0, ap=[[0, 128], [1, 128]]),
            )
            obs_sb = consts.tile([128, 4, BS], BF16)
            nc.sync.dma_start(out=obs_sb[:, :, :], in_=obs_T[:, :, :])

            # ---- gathers: 2 chunks per tile, 8 queues, issued up-front ----
            gth = []
            for t in range(NT):
                g = gat.tile([128, 896], BF16, tag=f"g{t}")
                nc.gpsimd.dma_gather(
                    out_ap=g[:, 0:640].rearrange("p (x n) -> p x n", x=1),
                    in_ap=comb_emb[t * 128 * R:(t + 1) * 128 * R, :],
                    idxs_ap=idx_sb[:, t, 0:40],
                    num_idxs=640, num_idxs_reg=640, elem_size=H,
                    transpose=True, queue_num=(t % 2) * 2 + (t // 2),
                )
                nc.gpsimd.dma_gather(
                    out_ap=g[:, 640:896].rearrange("p (x n) -> p x n", x=1),
                    in_ap=comb_emb[t * 128 * R:(t + 1) * 128 * R, :],
                    idxs_ap=idx_sb[:, t, 40:56],
                    num_idxs=256, num_idxs_reg=256, elem_size=H,
                    transpose=True, queue_num=(t % 2) * 2 + 1 - (t // 2),
                )
                gth.append(g)

            # ---- per-tile: shared (obs+sub), node matmuls, broadcast-add,
            # transposes. NOTE: each sh region's 5 accumulating matmuls must
            # be consecutive -- a start=True on the shared PSUM bank clears
            # has_written bits for the WHOLE bank. ----
            hbL = work.tile([128, NT, 768], BF16, tag="hbL")
            e_sd = small.tile([128, NT, 48], F32, tag="esd")
            sh_sb = consts.tile([128, 4, 128], BF16)
            sh_ps = psS.tile([128, 4, 128], F32)
            for t in range(NT):
                g = gth[t]
                # shared = Wfold^T obs^T + W1b^T sub^T (consecutive group)
                for c in range(4):
                    nc.tensor.matmul(
                        sh_ps[:, t, :], wfold_sb[:, c, :],
                        obs_sb[:, c, t * 128:(t + 1) * 128],
                        start=(c == 0), stop=False,
                    )
                nc.tensor.matmul(sh_ps[:, t, :], w1bc_sb[:, 0, :],
                                 g[:, 512:640], start=False, stop=True)
                nc.scalar.activation(sh_sb[:, t, :], sh_ps[:, t, :], ACT.Copy)
                h_ps = psH.tile([128, 768], F32, tag="hps")
                # node part (W1c stationary)
                nc.tensor.matmul(h_ps[:, 0:512], w1bc_sb[:, 1, :], g[:, 0:512],
                                 start=True, stop=False)
                nc.tensor.matmul(h_ps[:, 512:768], w1bc_sb[:, 1, :],
                                 g[:, 640:896], start=True, stop=False)
                # broadcast shared over the 6 nodes (ident stationary,
                # 0-stride moving)
                for hs, nk in ((slice(0, 512), 4), (slice(512, 768), 2)):
                    mov = (sh_sb[:, t, :].unsqueeze(1)
                           .broadcast_to([128, nk, 128]))
                    nc.tensor.matmul(h_ps[:, hs], ident_sb[:, :], mov,
                                     start=False, stop=True)
                # h^T (+bias) -> SBUF bf16
                hT = htp.tile([128, 768], BF16, tag="hT")
                nc.scalar.activation(hT[:, :], h_ps[:, :], ACT.Identity,
                                     bias=biash_sb[:, :], scale=1.0)
                # fused transpose + e: out[:,k,0:128]=hbL block,
                # out[:,k,128:136]=e_src/e_dst. Split 3+3 so no matmul
                # output straddles a PSUM bank (136 f32 x 3 fits in 2KB).
                ps2a = psT.tile([128, 3, 136], F32, tag="ps2a")
                ps2b = psT.tile([128, 3, 136], F32, tag="ps2b")
                for k in range(K):
                    ps2k = ps2a[:, k, :] if k < 3 else ps2b[:, k - 3, :]
                    nc.tensor.matmul(
                        ps2k, hT[:, k * 128:(k + 1) * 128],
                        tri_sb[:, :], start=True, stop=True,
                    )
                # trident's perm already emits (f,h) order: plain evicts
                nc.scalar.activation(
                    hbL[:, t, 0:384].rearrange("p (k f) -> p k f", k=3),
                    ps2a[:, :, 0:128], ACT.Copy)
                nc.scalar.activation(
                    hbL[:, t, 384:768].rearrange("p (k f) -> p k f", k=3),
                    ps2b[:, :, 0:128], ACT.Copy)
                nc.vector.tensor_sub(
                    e_sd[:, t, 0:24].rearrange("p (k s) -> p k s", k=3),
                    ps2a[:, :, 128:136], ce_sb[:, 0:24].rearrange(
                        "p (k s) -> p k s", k=3))
                nc.vector.tensor_sub(
                    e_sd[:, t, 24:48].rearrange("p (k s) -> p k s", k=3),
                    ps2b[:, :, 128:136], ce_sb[:, 24:48].rearrange(
                        "p (k s) -> p k s", k=3))

            # ---- alpha + apply, pipelined over tile PAIRS so the vector
            # phase starts as soon as tiles 0/1 land. Layout (t,i,j,h);
            # apply muls in (j,f,h) (h-innermost step-1 -> DVE 2x);
            # three batched fold adds per half. ----
            e4 = e_sd[:, :, :].rearrange("p t (k s h) -> p t k s h", s=2, h=4)
            e_raw = small.tile([128, NT, 144], BF16, tag="eraw")
            e_lrf = small.tile([128, NT * 144], BF16, tag="elr")
            E1 = small.tile([128, NT * 144], F32, tag="E1")
            Z1 = small.tile([128, 96], F32, tag="Z1")
            rZ1 = small.tile([128, 96], F32, tag="rZ1")
            al = small.tile([128, NT, 144], BF16, tag="al")
            prod = work.tile([128, 24, 768], BF16, tag="prod")
            f1 = work.tile([128, 24, 384], BF16, tag="f1")
            f2 = work.tile([128, 24, 128], BF16, tag="f2")
            attn = work.tile([128, 24, 128], BF16, tag="attn")
            e_rawf = e_raw[:, :, :].rearrange("p t x -> p (t x)")
            for half in range(2):
                th0 = 2 * half
                for t in (th0, th0 + 1):
                    e_dst_ap = (e4[:, t, :, 1, :].unsqueeze(2)
                                .broadcast_to([128, 6, 6, 4]))
                    e_src_ap = (e4[:, t, :, 0, :].unsqueeze(1)
                                .broadcast_to([128, 6, 6, 4]))
                    nc.vector.tensor_tensor(
                        e_raw[:, t, :].rearrange("p (i j h) -> p i j h",
                                                 j=6, h=4),
                        e_dst_ap, e_src_ap, OP.add,
                    )
                sl = slice(th0 * 144, (th0 + 2) * 144)
                zs = slice(half * 48, (half + 1) * 48)
                nc.vector.scalar_tensor_tensor(
                    e_lrf[:, sl], e_rawf[:, sl], LRELU_SLOPE, e_rawf[:, sl],
                    OP.mult, OP.max)
                nc.scalar.activation(E1[:, sl], e_lrf[:, sl], ACT.Exp)
                nc.vector.tensor_reduce(
                    Z1[:, zs],
                    E1[:, sl].rearrange("p (ti j h) -> p ti j h", j=6, h=4)
                    .transpose([0, 1, 3, 2]),
                    axis=AX.X, op=OP.add)
                nc.vector.reciprocal_approx_fast(rZ1[:, zs], Z1[:, zs])
                nc.vector.tensor_tensor(
                    al[:, th0:th0 + 2, :].rearrange(
                        "p t (i j h) -> p (t i) j h", j=6, h=4),
                    E1[:, sl].rearrange("p (ti j h) -> p ti j h", j=6, h=4),
                    rZ1[:, zs].rearrange("p (ti h) -> p ti h", h=4)
                    .unsqueeze(2).broadcast_to([128, 12, 6, 4]),
                    OP.mult,
                )
                for i in range(K):
                    eng = nc.gpsimd if (GPSIMD_MULS and i >= 4) else nc.vector
                    for t in (th0, th0 + 1):
                        eng.tensor_tensor(
                            prod[:, t * 6 + i, :].rearrange(
                                "p (j f h) -> p j f h", f=32, h=4),
                            al[:, t, i * 24:(i + 1) * 24]
                            .rearrange("p (j h) -> p j h", h=4)
                            .unsqueeze(2).broadcast_to([128, 6, 32, 4]),
                            hbL[:, t, :].rearrange("p (j f h) -> p j f h",
                                                   f=32, h=4),
                            OP.mult,
                        )
                cs = slice(th0 * 6, (th0 + 2) * 6)
                nc.vector.tensor_add(f1[:, cs, :], prod[:, cs, 0:384],
                                     prod[:, cs, 384:768])
                nc.vector.tensor_add(f2[:, cs, :], f1[:, cs, 0:128],
                                     f1[:, cs, 128:256])
                nc.vector.tensor_add(attn[:, cs, :], f2[:, cs, :],
                                     f1[:, cs, 256:384])

            # ---- elu(x) = exp(min(x,0)) + relu(x) - 1 (-1 folded into c2)
            attn_f = attn[:, :, :].rearrange("p c f -> p (c f)")
            min_x = work.tile([128, NT * 768], BF16, tag="minx")
            exp_m = work.tile([128, NT * 768], BF16, tag="expm")
            relu_x = work.tile([128, NT * 768], BF16, tag="relux")
            v1 = work.tile([128, NT * 768], BF16, tag="v1")
            HB = NT * 384
            for hh in range(2):
                es = slice(hh * HB, (hh + 1) * HB)
                nc.vector.tensor_scalar_min(min_x[:, es], attn_f[:, es], 0.0)
                nc.scalar.activation(exp_m[:, es], min_x[:, es], ACT.Exp)
                nc.vector.tensor_scalar_max(relu_x[:, es], attn_f[:, es], 0.0)
                nc.vector.tensor_add(v1[:, es], exp_m[:, es], relu_x[:, es])

            if DEBUG_DUMP:
                # dbg1: e_sd (192) | Z1 (96) | rZ1 (96) | al (576, as f32)
                dbg1_sb = work.tile([128, 960], F32, tag="dbg1sb")
                nc.vector.tensor_copy(dbg1_sb[:, 0:192],
                                      e_sd[:, :, :].rearrange("p t x -> p (t x)"))
                nc.vector.tensor_copy(dbg1_sb[:, 192:288], Z1[:, :])
                nc.vector.tensor_copy(dbg1_sb[:, 288:384], rZ1[:, :])
                nc.vector.tensor_copy(dbg1_sb[:, 384:960],
                                      al[:, :, :].rearrange("p t x -> p (t x)"))
                nc.sync.dma_start(out=dbg1[:, :], in_=dbg1_sb[:, :])
                dbg2_sb = work.tile([128, 3072], F32, tag="dbg2sb")
                nc.vector.tensor_copy(dbg2_sb[:, :], attn_f)
                nc.sync.dma_start(out=dbg2[:, :], in_=dbg2_sb[:, :])

            # ---- layer 2: h2 = sum_f v1*W2 (2x mul, 2x fold adds, then a
            # small reduce) ----
            vw = work.tile([128, 24, 128], BF16, tag="vw")
            nc.vector.tensor_tensor(
                vw[:, :, :],
                v1[:, :].rearrange("p (c f) -> p c f", f=128),
                w2_sb[:, :].unsqueeze(1).broadcast_to([128, 24, 128]),
                OP.mult)
            vwa = work.tile([128, 24, 64], BF16, tag="vwa")
            nc.vector.tensor_add(vwa[:, :, :], vw[:, :, 0:64],
                                 vw[:, :, 64:128])
            vwb = work.tile([128, 24, 32], F32, tag="vwb")
            nc.vector.tensor_add(vwb[:, :, :], vwa[:, :, 0:32],
                                 vwa[:, :, 32:64])
            h2 = small.tile([128, 24], F32, tag="h2")
            nc.vector.tensor_reduce(h2[:, :], vwb[:, :, :], axis=AX.X,
                                    op=OP.add)
            h2c = small.tile([128, 24], F32, tag="h2c")
            nc.vector.tensor_scalar(h2c[:, :], h2[:, :], -c2, None, OP.add)
            h2s = small.tile([128, 24], F32, tag="h2s")
            nc.vector.tensor_scalar(h2s[:, :], h2[:, :], as2,
                                    -c2 * (as2 + ad2), OP.mult, OP.add)
            # e2 = ad2*h2[i] + (as2*h2[j] - c2*(as2+ad2)); layout (t, i, j)
            h2d = small.tile([128, 24], F32, tag="h2d")
            nc.vector.tensor_scalar(h2d[:, :], h2[:, :], ad2, None, OP.mult)
            h2dv = h2d[:, :].rearrange("p (t i) -> p t i", t=4)
            h2sv = h2s[:, :].rearrange("p (t j) -> p t j", t=4)
            h2cv = h2c[:, :].rearrange("p (t j) -> p t j", t=4)
            e2_raw = small.tile([128, 144], F32, tag="e2raw")
            nc.vector.tensor_tensor(
                e2_raw[:, :].rearrange("p (t i j) -> p t i j", t=4, j=6),
                h2dv.unsqueeze(3).broadcast_to([128, 4, 6, 6]),
                h2sv.unsqueeze(2).broadcast_to([128, 4, 6, 6]),
                OP.add,
            )
            e2_lr = small.tile([128, 144], F32, tag="e2lr")
            nc.vector.scalar_tensor_tensor(
                e2_lr[:, :], e2_raw[:, :], LRELU_SLOPE, e2_raw[:, :],
                OP.mult, OP.max)
            E2 = small.tile([128, 144], F32, tag="E2")
            nc.scalar.activation(E2[:, :], e2_lr[:, :], ACT.Exp)
            E2v = E2[:, :].rearrange("p (ti j) -> p ti j", j=6)
            Z2 = small.tile([128, 24], F32, tag="Z2")
            nc.vector.tensor_reduce(Z2[:, :], E2v, axis=AX.X, op=OP.add)
            rZ2 = small.tile([128, 24], F32, tag="rZ2")
            nc.vector.reciprocal_approx_fast(rZ2[:, :], Z2[:, :])
            P2 = small.tile([128, 144], F32, tag="P2")
            nc.vector.tensor_tensor(
                P2[:, :].rearrange("p (t i j) -> p t i j", t=4, j=6),
                E2[:, :].rearrange("p (t i j) -> p t i j", t=4, j=6),
                h2cv.unsqueeze(2).broadcast_to([128, 4, 6, 6]),
                OP.mult,
            )
            S2 = small.tile([128, 24], F32, tag="S2")
            nc.vector.tensor_reduce(
                S2[:, :], P2[:, :].rearrange("p (ti j) -> p ti j", j=6),
                axis=AX.X, op=OP.add,
            )
            out_sb = small.tile([128, 24], F32, tag="outsb")
            nc.vector.tensor_mul(out_sb[:, :], S2[:, :], rZ2[:, :])
            if b2 != 0.0:
                nc.vector.tensor_scalar(out_sb[:, :], out_sb[:, :], b2,
                                        None, OP.add)
            nc.sync.dma_start(
                out=bass.AP(tensor=out_ext, offset=0,
                            ap=[[K, 128], [128 * K, NT], [1, K]]),
                in_=out_sb[:, :].rearrange("p (t k) -> p t k", t=NT))

    nc.finalize()
    return nc


def prep_core_inputs(core, org_obs, node_embeddings, substation_embeddings,
                     sub_choice, sub_id_to_elem_id, W_proj, b_proj, W1,
                     a_src1, a_dst1, b1, W2, a_src2, a_dst2, b2):
    """Host-side shard + layout prep for one core (index math and weight
    folding only -- all tensor FLOPs on the batch stay on device)."""
    bf = ml_dtypes.bfloat16
    s = slice(core * BS, (core + 1) * BS)

    obs = np.asarray(org_obs[s], np.float32)
    obs_T = np.zeros((OBS_PAD, BS), np.float32)
    obs_T[:OBS, :] = obs.T
    obs_T4 = obs_T.reshape(4, 128, BS).transpose(1, 0, 2).copy()

    comb = np.concatenate(
        [np.asarray(node_embeddings[s], np.float32),
         np.asarray(substation_embeddings[s], np.float32)], axis=1
    ).reshape(BS * R, H).astype(bf)

    sub_idx = np.asarray(sub_choice[s, 0], np.int64)
    elem = np.asarray(sub_id_to_elem_id, np.int64)[sub_idx]   # [BS, K]

    idx_comb = np.zeros((NT, 128, 56), np.int16)
    for t in range(NT):
        bloc = np.arange(128)
        el = elem[t * 128:(t + 1) * 128]
        Ln = (bloc[None, :] * R + el.T)                        # [K, 128]
        Ls = bloc * R + N + sub_idx[t * 128:(t + 1) * 128]
        L = np.concatenate([Ln[:4].reshape(-1), Ls,
                            Ln[4:].reshape(-1)])               # 896
        blk = L.reshape(56, 16).T.astype(np.int16)
        idx_comb[t] = np.tile(blk, (8, 1))
    idx_comb = np.ascontiguousarray(
        idx_comb.transpose(1, 0, 2).reshape(128, NT * 56))

    W1 = np.asarray(W1, np.float32)
    W1a, W1b, W1c = W1[0:H], W1[H:2 * H], W1[2 * H:3 * H]
    Wp = np.asarray(W_proj, np.float32)
    wfold = np.zeros((OBS_PAD, H), np.float32)
    wfold[:OBS] = Wp @ W1a
    wfold4 = wfold.reshape(4, 128, H).transpose(1, 0, 2).copy()
    bias_h = np.asarray(b1, np.float32) + np.asarray(b_proj, np.float32) @ W1a

    asrc_m = np.zeros((H, 8), np.float32)
    for h in range(HEADS):
        asrc_m[h * FH:(h + 1) * FH, h] = a_src1[h]
        asrc_m[h * FH:(h + 1) * FH, 4 + h] = a_dst1[h]
    # first 128 cols: permutation (h,f)->(f,h) so the transpose emits the
    # apply-friendly layout directly
    perm = np.zeros((H, H), np.float32)
    for h in range(HEADS):
        for f in range(FH):
            perm[h * FH + f, f * HEADS + h] = 1.0
    trident = np.concatenate([perm, asrc_m], axis=1)

    # e must be computed from h WITHOUT b1 (but WITH the obs-projection
    # bias, which is part of the reference h) -> correct only for b1.
    bh = np.asarray(b1, np.float32).reshape(HEADS, FH)
    cek = np.zeros((K, 8), np.float32)
    cek[:, 0:4] = (bh * np.asarray(a_src1, np.float32)).sum(-1)[None, :]
    cek[:, 4:8] = (bh * np.asarray(a_dst1, np.float32)).sum(-1)[None, :]

    return {
        "obs_T": obs_T4.astype(bf),
        "comb_emb": comb,
        "idx_comb": idx_comb,
        "wfold": wfold4.astype(bf),
        "w1bc": np.stack([W1b, W1c], axis=1).astype(bf),
        "trident": trident.astype(bf),
        "biash": bias_h.reshape(H, 1).astype(np.float32),
        "ce": cek.reshape(1, 48).astype(np.float32),
        # W2 permuted to the (f,h) feature order used by the apply layout
        "w2r": np.asarray(W2, np.float32).reshape(HEADS, FH).T
               .reshape(1, H).astype(bf),
        "identd": np.eye(H, dtype=np.float32).astype(bf),
    }


_GRAPH_CACHE = {}
LAST_RESULTS = None


def kernel(**inputs):
    inp = {k: np.asarray(v) for k, v in inputs.items()}
    W2 = np.asarray(inp["W2"], np.float32)
    scalars = {
        "a_src2": float(np.asarray(inp["a_src2"]).reshape(-1)[0]),
        "a_dst2": float(np.asarray(inp["a_dst2"]).reshape(-1)[0]),
        "b2": float(np.asarray(inp["b2"]).reshape(-1)[0]),
        "c2": float(W2.sum()),
    }
    key = tuple(sorted(scalars.items()))
    if key not in _GRAPH_CACHE:
        _GRAPH_CACHE[key] = build_graph(scalars)
    nc = _GRAPH_CACHE[key]

    in_maps = [
        prep_core_inputs(
            c, inp["org_obs"], inp["node_embeddings"],
            inp["substation_embeddings"], inp["sub_choice"],
            inp["sub_id_to_elem_id"], inp["W_proj"], inp["b_proj"], inp["W1"],
            inp["a_src1"], inp["a_dst1"], inp["b1"], inp["W2"], inp["a_src2"],
            inp["a_dst2"], inp["b2"],
        )
        for c in range(NCORES)
    ]
    res = run_bass_kernel_spmd(nc, in_maps, core_ids=list(range(NCORES)))
    global LAST_RESULTS
    LAST_RESULTS = res
    out = np.concatenate([res.results[c]["out"] for c in range(NCORES)], axis=0)
    return out.reshape(B, K, 1).astype(np.float32)


if __name__ == "__main__":
    g = build_graph({"a_src2": 0.01, "a_dst2": 0.02, "b2": 0.0, "c2": 0.1})
    print("graph built ok")
